# revision 1
# baseline (speedup 1.0000x reference)
"""Trainium2 Bass kernel for a cross-attention transformer block.

Reference computation (per batch element b of 8):
    kv = img_feat + img_pos                       [4096, 512]
    q = pattern @ Wq + bq                         [1024, 512]
    k = kv @ Wk + bk ; v = kv @ Wv + bv           [4096, 512]
    scores = q @ k.T                              [1024, 4096]
    attn = softmax(scores, axis=QUERY)            (normalized over the 1024 axis)
    x = attn @ v                                  [1024, 512]
    h = LN(pattern + x) ; out = LN(h + relu(h@W1+b1)@W2 + b2)

Sharding: pure data-parallel — batch 8 across 8 NeuronCores, one batch
element per core, no collectives.

Per-core strategy: activations kept feature-on-partition ("T" layout) for
matmuls contracting over features. q/k/scores run as float32r (fp32 data,
1 cycle/row on the PE at moving dim >= 256); attn/v/FFN run in bf16.
Softmax over the query axis is a free-axis reduce in the scores^T
[kv_token_partition, query_free] layout; the softmax normalization is
folded into v (v''[n] = v[n]/denom[n]). Unnormalized attn chunks spill to
DRAM so the whole attn@v contraction runs as a second pass with all eight
PSUM banks accumulating, which leaves the scores pipeline in pass one
fully double-buffered.
"""

import numpy as np

P = 128
C = 512          # feature dim
CK = C // P      # 4 feature chunks
M = 1024         # queries
MT = M // P      # 8 query tiles
N = 4096         # kv tokens
NT = 8           # n-tiles of 512
NC = N // P      # 32 kv chunks
D = 2048         # ffn hidden
DC = D // P      # 16 ffn chunks
B = 8            # batch == cores
EPS = 1e-5

_CACHE = {}


def _build():
    from contextlib import ExitStack

    import concourse.bacc as bacc
    import concourse.bass as bass
    import concourse.mybir as mybir
    import concourse.tile as tile
    from concourse.masks import make_identity

    f32 = mybir.dt.float32
    bf16 = mybir.dt.bfloat16
    f32r = mybir.dt.float32r
    Alu = mybir.AluOpType
    Act = mybir.ActivationFunctionType
    AX = mybir.AxisListType

    nc = bacc.Bacc("TRN2", target_bir_lowering=False, debug=False, num_devices=B)

    img_feat = nc.dram_tensor("img_feat", (N, C), f32, kind="ExternalInput")
    img_pos = nc.dram_tensor("img_pos", (N, C), f32, kind="ExternalInput")
    pattern = nc.dram_tensor("pattern_feat", (M, C), f32, kind="ExternalInput")
    Wq = nc.dram_tensor("Wq", (C, C), f32, kind="ExternalInput")
    bq = nc.dram_tensor("bq", (C,), f32, kind="ExternalInput")
    Wk = nc.dram_tensor("Wk", (C, C), f32, kind="ExternalInput")
    bk = nc.dram_tensor("bk", (C,), f32, kind="ExternalInput")
    Wv = nc.dram_tensor("Wv", (C, C), f32, kind="ExternalInput")
    bv = nc.dram_tensor("bv", (C,), f32, kind="ExternalInput")
    ln1_g = nc.dram_tensor("ln1_g", (C,), f32, kind="ExternalInput")
    ln1_b = nc.dram_tensor("ln1_b", (C,), f32, kind="ExternalInput")
    W1 = nc.dram_tensor("W1", (C, D), f32, kind="ExternalInput")
    b1 = nc.dram_tensor("b1", (D,), f32, kind="ExternalInput")
    W2 = nc.dram_tensor("W2", (D, C), f32, kind="ExternalInput")
    b2 = nc.dram_tensor("b2", (C,), f32, kind="ExternalInput")
    ln2_g = nc.dram_tensor("ln2_g", (C,), f32, kind="ExternalInput")
    ln2_b = nc.dram_tensor("ln2_b", (C,), f32, kind="ExternalInput")
    out = nc.dram_tensor("out", (M, C), f32, kind="ExternalOutput")

    def bcast(handle):
        """AP replicating a [C]-shaped dram vector across all partitions."""
        ap = handle[:]
        return bass.AP(tensor=ap.tensor, offset=ap.offset, ap=[[0, P], *ap.ap])

    with tile.TileContext(nc) as tc, ExitStack() as top:
        const = top.enter_context(tc.tile_pool(name="const", bufs=1))
        dram = top.enter_context(tc.tile_pool(name="dram", bufs=1, space="DRAM"))

        # ---- constants -------------------------------------------------
        ident = const.tile([P, P], f32)
        make_identity(nc, ident)
        eps_t = const.tile([P, 1], f32)
        nc.vector.memset(eps_t, EPS)
        bq_t = const.tile([P, CK], f32)
        nc.scalar.dma_start(bq_t, bq[:].rearrange("(c p) -> p c", p=P))
        bk_t = const.tile([P, CK], f32)
        nc.scalar.dma_start(bk_t, bk[:].rearrange("(c p) -> p c", p=P))
        b1_t = const.tile([P, DC], f32)
        nc.scalar.dma_start(b1_t, b1[:].rearrange("(c p) -> p c", p=P))
        bv_bc = const.tile([P, C], f32)
        nc.gpsimd.dma_start(bv_bc, bcast(bv))
        b2_bc = const.tile([P, C], f32)
        nc.gpsimd.dma_start(b2_bc, bcast(b2))
        g1_bc = const.tile([P, C], f32)
        nc.gpsimd.dma_start(g1_bc, bcast(ln1_g))
        b1ln_bc = const.tile([P, C], f32)
        nc.gpsimd.dma_start(b1ln_bc, bcast(ln1_b))
        g2_bc = const.tile([P, C], f32)
        nc.gpsimd.dma_start(g2_bc, bcast(ln2_g))
        b2ln_bc = const.tile([P, C], f32)
        nc.gpsimd.dma_start(b2ln_bc, bcast(ln2_b))

        # unnormalized attn (exp) spilled per chunk
        attn_dram = dram.tile([NC, P, M], bf16)

        # long-lived sbuf
        h_pool = top.enter_context(tc.tile_pool(name="hp", bufs=1))
        h_sb = h_pool.tile([P, MT, C], f32)
        sC = top.enter_context(tc.tile_pool(name="sC", bufs=1))
        pat_tm = sC.tile([P, MT, C], f32)
        vpool = top.enter_context(tc.tile_pool(name="vp", bufs=NC))

        v_tiles = []

        with ExitStack() as mid:
            sB = mid.enter_context(tc.tile_pool(name="sB", bufs=1))
            io = mid.enter_context(tc.tile_pool(name="io", bufs=2))
            kvc = mid.enter_context(tc.tile_pool(name="kvc", bufs=3))
            kvp = mid.enter_context(tc.tile_pool(name="kvp", bufs=2))
            att = mid.enter_context(tc.tile_pool(name="att", bufs=3))
            sm = mid.enter_context(tc.tile_pool(name="sm", bufs=4))
            tp_ps = mid.enter_context(tc.tile_pool(name="tp_ps", bufs=2, space="PSUM"))
            mm_ps = mid.enter_context(tc.tile_pool(name="mm_ps", bufs=2, space="PSUM"))
            sc_ps = mid.enter_context(tc.tile_pool(name="sc_ps", bufs=2, space="PSUM"))

            # ---- pattern load (per tile) + patT + qT -------------------
            with ExitStack() as pro:
                sA = pro.enter_context(tc.tile_pool(name="sA", bufs=1))
                patT = sA.tile([P, CK, M], f32r)
                for mt in range(MT):
                    nc.sync.dma_start(pat_tm[:, mt, :],
                                      pattern[mt * P:(mt + 1) * P, :])
                    tp = tp_ps.tile([P, CK, P], f32, tag="tp")
                    for ci in range(CK):
                        nc.tensor.transpose(
                            tp[:, ci, :], pat_tm[:, mt, ci * P:(ci + 1) * P],
                            ident)
                    if mt % 2 == 0:
                        nc.vector.tensor_copy(
                            patT[:, :, mt * P:(mt + 1) * P], tp)
                    else:
                        nc.scalar.copy(
                            patT[:, :, mt * P:(mt + 1) * P], tp)

                with ExitStack() as wst_scope:
                    wstp = wst_scope.enter_context(
                        tc.tile_pool(name="wstp", bufs=1))
                    Wq_sb = sB.tile([P, CK, C], f32r)
                    Wk_sb = sB.tile([P, CK, C], f32r)
                    Wv_sb = sB.tile([P, CK, C], f32r)
                    for i, (dst, src) in enumerate(
                            ((Wq_sb, Wq), (Wk_sb, Wk), (Wv_sb, Wv))):
                        wst = wstp.tile([P, CK, C], f32, tag="wst")
                        nc.sync.dma_start(
                            wst, src[:, :].rearrange("(c p) n -> p c n", p=P))
                        if i % 2 == 0:
                            nc.scalar.copy(dst, wst)
                        else:
                            nc.vector.tensor_copy(dst, wst)

                qT = sB.tile([P, CK, M], f32r)
                for co in range(CK):
                    for mh in range(2):
                        ps = mm_ps.tile([P, 512], f32, tag="mm")
                        for ci in range(CK):
                            nc.tensor.matmul(
                                ps, Wq_sb[:, ci, co * P:(co + 1) * P],
                                patT[:, ci, mh * 512:(mh + 1) * 512],
                                start=(ci == 0), stop=(ci == CK - 1))
                        nc.scalar.activation(
                            qT[:, co, mh * 512:(mh + 1) * 512], ps, Act.Identity,
                            bias=bq_t[:, co:co + 1])

            # ============ pass 1: projections + scores + softmax ========
            for t in range(NT):
                # batched img loads: two 256-token halves per 512-token tile
                ifh = [io.tile([P, 2, C], f32, tag="if", name=f"if{t}_{h}")
                       for h in range(2)]
                iph = [io.tile([P, 2, C], f32, tag="ip", name=f"ip{t}_{h}")
                       for h in range(2)]
                for h in range(2):
                    rows = (t * 4 + h * 2) * P
                    nc.sync.dma_start(
                        ifh[h], img_feat[rows:rows + 2 * P, :].rearrange(
                            "(c p) n -> p c n", p=P))
                    nc.sync.dma_start(
                        iph[h], img_pos[rows:rows + 2 * P, :].rearrange(
                            "(c p) n -> p c n", p=P))
                kvT_t = kvp.tile([P, CK, 512], f32r, tag="kvT")
                for ncc in range(4):
                    kvt = kvc.tile([P, C], f32, tag="kv")
                    nc.gpsimd.tensor_add(
                        kvt, ifh[ncc // 2][:, ncc % 2, :], iph[ncc // 2][:, ncc % 2, :])
                    tp = tp_ps.tile([P, CK, P], f32, tag="tp")
                    for ci in range(CK):
                        nc.tensor.transpose(
                            tp[:, ci, :], kvt[:, ci * P:(ci + 1) * P], ident)
                    if ncc % 2 == 0:
                        nc.vector.tensor_copy(
                            kvT_t[:, :, ncc * P:(ncc + 1) * P], tp)
                    else:
                        nc.scalar.copy(
                            kvT_t[:, :, ncc * P:(ncc + 1) * P], tp)

                # k^T tile [C-part, 512] with bias
                kT_t = kvp.tile([P, CK, 512], f32r, tag="kT")
                for co in range(CK):
                    ps = mm_ps.tile([P, 512], f32, tag="mm")
                    for ci in range(CK):
                        nc.tensor.matmul(
                            ps, Wk_sb[:, ci, co * P:(co + 1) * P], kvT_t[:, ci, :],
                            start=(ci == 0), stop=(ci == CK - 1))
                    nc.vector.tensor_scalar(
                        kT_t[:, co, :], ps, bk_t[:, co:co + 1], None, op0=Alu.add)

                # v chunks [token-part, C] bf16 with bias
                for ncc in range(4):
                    ps = mm_ps.tile([P, 512], f32, tag="mm")
                    for ci in range(CK):
                        nc.tensor.matmul(
                            ps, kvT_t[:, ci, ncc * P:(ncc + 1) * P], Wv_sb[:, ci, :],
                            start=(ci == 0), stop=(ci == CK - 1))
                    vt = vpool.tile([P, C], bf16, tag="v")
                    nc.vector.tensor_tensor(vt, ps, bv_bc, op=Alu.add)
                    v_tiles.append(vt)

                # scores + softmax per chunk
                for ncc in range(4):
                    j = t * 4 + ncc
                    ps_s = sc_ps.tile([P, M], f32, tag="sc")
                    for mh in range(2):
                        for ci in range(CK):
                            nc.tensor.matmul(
                                ps_s[:, mh * 512:(mh + 1) * 512],
                                kT_t[:, ci, ncc * P:(ncc + 1) * P],
                                qT[:, ci, mh * 512:(mh + 1) * 512],
                                start=(ci == 0), stop=(ci == CK - 1))
                    negmax = sm.tile([P, 1], f32, tag="nm")
                    nc.vector.tensor_reduce(
                        negmax, ps_s[:, :], axis=AX.X, op=Alu.max, negate=True)
                    sums = sm.tile([P, 2], f32, tag="sums")
                    attn_t = att.tile([P, M], bf16, tag="a")
                    nc.scalar.activation(attn_t[:, :512], ps_s[:, :512], Act.Exp,
                                         bias=negmax, accum_out=sums[:, 0:1])
                    nc.scalar.activation(attn_t[:, 512:], ps_s[:, 512:], Act.Exp,
                                         bias=negmax, accum_out=sums[:, 1:2])
                    nc.sync.dma_start(attn_dram[j], attn_t)
                    denom = sm.tile([P, 1], f32, tag="dn")
                    nc.vector.tensor_reduce(denom, sums, axis=AX.X, op=Alu.add)
                    rcp = sm.tile([P, 1], f32, tag="rcp")
                    nc.vector.reciprocal(rcp, denom)
                    # fold softmax normalization into v
                    nc.vector.tensor_scalar_mul(v_tiles[j], v_tiles[j], rcp)

        # ---- FFN weights (DMA overlaps pass 2) -------------------------
        wpool = top.enter_context(tc.tile_pool(name="wp", bufs=1))
        wload = top.enter_context(tc.tile_pool(name="wl", bufs=1))
        lnp = top.enter_context(tc.tile_pool(name="ln", bufs=3))
        outp = top.enter_context(tc.tile_pool(name="outp", bufs=3))

        W1_sb = wpool.tile([P, CK, D], bf16)
        W2_sb = wpool.tile([P, DC, C], bf16)

        def load_ffn_weights():
            for ci in range(CK):
                for dh in range(2):
                    wt = wload.tile([P, D // 2], f32, tag="w1l",
                                    name=f"w1l{ci}_{dh}")
                    nc.scalar.dma_start(
                        wt, W1[:, dh * (D // 2):(dh + 1) * (D // 2)].rearrange(
                            "(c p) d -> p c d", p=P)[:, ci, :])
                    nc.gpsimd.tensor_copy(
                        W1_sb[:, ci, dh * (D // 2):(dh + 1) * (D // 2)], wt)
            for dc2 in range(8):
                wt = wload.tile([P, 2, C], f32, tag="w2l", name=f"w2l{dc2}")
                nc.scalar.dma_start(
                    wt, W2[:, :].rearrange(
                        "(c p) n -> p c n", p=P)[:, dc2 * 2:(dc2 + 1) * 2, :])
                nc.gpsimd.tensor_copy(W2_sb[:, dc2 * 2:(dc2 + 1) * 2, :], wt)

        def ln_apply(src_psum, resid, g_bc, bln_bc, extra_bc, dst):
            """dst = LN(src + resid [+ extra]) * g + b   (dst/resid in SBUF)"""
            tpre = lnp.tile([P, C], f32, tag="tpre")
            nc.vector.tensor_tensor(tpre, src_psum, resid, op=Alu.add)
            if extra_bc is not None:
                nc.gpsimd.tensor_add(tpre, tpre, extra_bc)
            stats = lnp.tile([P, 6], f32, tag="st")
            nc.vector.bn_stats(stats, tpre)
            mv = lnp.tile([P, 2], f32, tag="mv")
            nc.vector.bn_aggr(mv, stats)
            sd = lnp.tile([P, 1], f32, tag="sd")
            nc.scalar.activation(sd, mv[:, 1:2], Act.Sqrt, bias=eps_t)
            rstd = lnp.tile([P, 1], f32, tag="rs")
            nc.vector.reciprocal(rstd, sd)
            xc = lnp.tile([P, C], f32, tag="xc")
            nc.vector.tensor_scalar(xc, tpre, mv[:, 0:1], rstd,
                                    op0=Alu.subtract, op1=Alu.mult)
            nc.gpsimd.tensor_mul(xc, xc, g_bc)
            nc.gpsimd.tensor_add(dst, xc, bln_bc)

        # ============ pass 2: x = attn'' @ v'' , then LN1 ===============
        # Chunks 0..7 stream chunk-major from DRAM (feeds the PE while the
        # rest of attn loads); chunks 8..31 come from SBUF-resident attn,
        # walked in four 2-query-tile groups so each group's LN1 (and the
        # FFN behind it) overlaps the remaining x matmuls.
        NRES = NC - 8  # chunks resident in SBUF
        with ExitStack() as p2:
            x_ps_pool = p2.enter_context(
                tc.tile_pool(name="x_ps", bufs=8, space="PSUM"))
            tail_rd = p2.enter_context(tc.tile_pool(name="tr", bufs=4))
            atp = p2.enter_context(tc.tile_pool(name="atp", bufs=1))
            attn_sb = atp.tile([P, NRES, M], bf16)
            ats = []
            for j in range(8):
                at = tail_rd.tile([P, M], bf16, tag="at", name=f"at{j}")
                nc.sync.dma_start(at, attn_dram[j])
                ats.append(at)
            for jj in range(NRES):
                nc.sync.dma_start(attn_sb[:, jj, :], attn_dram[8 + jj])
            x_ps = [x_ps_pool.tile([P, C], f32, tag="x", name=f"xps{i}")
                    for i in range(MT)]
            for j in range(8):
                for ms in range(MT):
                    nc.tensor.matmul(
                        x_ps[ms], ats[j][:, ms * P:(ms + 1) * P], v_tiles[j],
                        start=(j == 0), stop=False)
            for grp in range(4):
                for ms in (grp * 2, grp * 2 + 1):
                    for jj in range(NRES):
                        nc.tensor.matmul(
                            x_ps[ms], attn_sb[:, jj, ms * P:(ms + 1) * P],
                            v_tiles[8 + jj],
                            start=False, stop=(jj == NRES - 1))
                for ms in (grp * 2, grp * 2 + 1):
                    ln_apply(x_ps[ms], pat_tm[:, ms, :], g1_bc, b1ln_bc, None,
                             h_sb[:, ms, :])
                if grp == 0:
                    load_ffn_weights()

        # ---- h^T (bf16) + FFN ------------------------------------------
        hq = top.enter_context(tc.tile_pool(name="hq", bufs=1))
        htp_ps = top.enter_context(tc.tile_pool(name="htp", bufs=2, space="PSUM"))
        y1_ps = top.enter_context(tc.tile_pool(name="y1ps", bufs=2, space="PSUM"))
        x2_ps = top.enter_context(tc.tile_pool(name="x2ps", bufs=2, space="PSUM"))

        hT = hq.tile([P, CK, M], bf16)
        for mt in range(MT):
            tp = htp_ps.tile([P, CK, P], f32, tag="htp")
            for ci in range(CK):
                nc.tensor.transpose(
                    tp[:, ci, :], h_sb[:, mt, ci * P:(ci + 1) * P], ident)
            nc.vector.tensor_copy(hT[:, :, mt * P:(mt + 1) * P], tp)

        y1T = hq.tile([P, DC, M], bf16)
        for dc in range(DC):
            for mh in range(2):
                ps = y1_ps.tile([P, 512], f32, tag="y1")
                for ci in range(CK):
                    nc.tensor.matmul(
                        ps, W1_sb[:, ci, dc * P:(dc + 1) * P],
                        hT[:, ci, mh * 512:(mh + 1) * 512],
                        start=(ci == 0), stop=(ci == CK - 1))
                nc.scalar.activation(
                    y1T[:, dc, mh * 512:(mh + 1) * 512], ps, Act.Relu,
                    bias=b1_t[:, dc:dc + 1])

        for mt in range(MT):
            ps = x2_ps.tile([P, 512], f32, tag="x2")
            for dc in range(DC):
                nc.tensor.matmul(
                    ps, y1T[:, dc, mt * P:(mt + 1) * P], W2_sb[:, dc, :],
                    start=(dc == 0), stop=(dc == DC - 1))
            ot = outp.tile([P, C], f32, tag="ot")
            ln_apply(ps, h_sb[:, mt, :], g2_bc, b2ln_bc, b2_bc, ot)
            nc.sync.dma_start(out[mt * P:(mt + 1) * P, :], ot)

    nc.finalize()
    return nc


def _get_nc():
    if "nc" not in _CACHE:
        _CACHE["nc"] = _build()
    return _CACHE["nc"]


def kernel(**inputs):
    from concourse import bass_utils

    nc = _get_nc()
    full = {k: np.ascontiguousarray(np.asarray(v, dtype=np.float32))
            for k, v in inputs.items()}
    in_maps = []
    for i in range(B):
        m = {
            "img_feat": full["img_feat"][i],
            "img_pos": full["img_pos"][i],
            "pattern_feat": full["pattern_feat"][i],
        }
        for w in ("Wq", "bq", "Wk", "bk", "Wv", "bv", "ln1_g", "ln1_b",
                  "W1", "b1", "W2", "b2", "ln2_g", "ln2_b"):
            m[w] = full[w]
        in_maps.append(m)
    res = bass_utils.run_bass_kernel_spmd(nc, in_maps, core_ids=list(range(B)))
    return np.stack([res.results[i]["out"] for i in range(B)], axis=0)

